# revision 18
# baseline (speedup 1.0000x reference)
"""Trainium2 Bass kernel for DecoderMultiHeadAttention.

Problem: B=2, S=2048, E=1024, H=16, D=O=64 multi-head attention with
per-head Q/K/V/out projections, outer-product query/key masking, and a
final (H*O) x E output linear.

Sharding (8 NeuronCores): core c owns batch b = c // 4 and head group
hg = c % 4 (4 heads). Each core computes the partial contribution of its
4 heads to its batch's final output (x_heads @ Wf_rows); the host sums
the 4 per-batch partials and adds the folded bias (bf + bo_flat @ Wf).

Device math per core (matmuls bf16, fp32 PSUM accumulation):
  q^T[d,s] = (Wq_aug^T @ [x_q; ones])            (d on partitions)
  k^T[d,s] = same
  v[t,d]   = ([x_v; ones]^T chunks as lhsT) @ Wv_aug, t on partitions
  scores^T[t,s] = k^T.T @ q^T  (K=64; the pair's 2 heads run concurrently
                  via PE row tile_position 0/64 into the two banks of one
                  [128,1024] PSUM tile)
  e^T = exp(scores^T/8 + kmask_bias[t])   (ONE ACT op per t-tile covers
                  both heads; bias is per-partition and head-independent)
  o_un^T[65,s] = [v | ones_col].T @ e^T-half     (row 64 = denominator)
  r[s] = qmask[s] / denom[s]; broadcast via K=1 fp32 matmul with ones lhsT
  o_n^T = o_un^T * r_bcast                       (DVE, SBUF-staged o_un)
  o2^T[o,s] = Wo.T @ o_n^T  (pair col-packed to psum rows 0-63/64-127
                             => x^T chunk directly)
  partial[s,:] = x^T.T @ Wf                      (final linear, no bias)

Projections run chunk-outer over 8 live PSUM accumulation groups so the
PE consumes each streamed input chunk at DMA line rate.
"""

import os
import sys

import numpy as np

if "/opt/trn_rl_repo" not in sys.path:
    sys.path.insert(0, "/opt/trn_rl_repo")

import ml_dtypes  # noqa: E402

import concourse.bass as bass  # noqa: E402
import concourse.tile as tile  # noqa: E402
from concourse import bacc, mybir  # noqa: E402
from concourse.bass_utils import run_bass_kernel_spmd  # noqa: E402

BF16 = ml_dtypes.bfloat16

B, S, E, H, D, O = 2, 2048, 1024, 16, 64, 64
HG = 4  # heads per core
N_CORES = 8
SBLK = 512  # s block for attention
TT = 128  # t tile (contraction chunk for attention)
NE = E // 128  # 8 e-chunks for projections
NSB = S // SBLK  # 4
NTT = S // TT  # 16
NST = S // 128  # 16 s tiles for the final linear
NEG = -30000.0  # additive mask bias (exp underflows to exactly 0)

_PROGRAM = None
LAST_RESULTS = None


def _build_program():
    dt = mybir.dt
    # Bacc (not raw Bass): its compile() runs generate_event_semaphores,
    # which legalizes multi-wait instructions down to the TRN2 limit of
    # one sync wait per instruction (walrus rejects more).
    nc = bacc.Bacc(
        "TRN2",
        target_bir_lowering=False,
        debug=False,
        enable_asserts=True,
        num_devices=N_CORES,
    )

    xq = nc.declare_dram_parameter("xq", [E, S], dt.bfloat16, isOutput=False)
    xk = nc.declare_dram_parameter("xk", [E, S], dt.bfloat16, isOutput=False)
    xv = nc.declare_dram_parameter("xv", [E, S], dt.bfloat16, isOutput=False)
    wq = nc.declare_dram_parameter("wq", [E + 1, HG * D], dt.bfloat16, isOutput=False)
    wk = nc.declare_dram_parameter("wk", [E + 1, HG * D], dt.bfloat16, isOutput=False)
    wv = nc.declare_dram_parameter("wv", [E + 1, HG * D], dt.bfloat16, isOutput=False)
    wo = nc.declare_dram_parameter("wo", [D, HG * O], dt.bfloat16, isOutput=False)
    wf = nc.declare_dram_parameter("wf", [HG * O, E], dt.bfloat16, isOutput=False)
    kbias = nc.declare_dram_parameter("kbias", [TT, NTT], dt.float32, isOutput=False)
    qmask = nc.declare_dram_parameter("qmask", [1, S], dt.float32, isOutput=False)
    out = nc.declare_dram_parameter("out", [S, E], dt.float32, isOutput=True)

    with tile.TileContext(nc) as tc:
        _emit(nc, tc, xq, xk, xv, wq, wk, wv, wo, wf, kbias, qmask, out)
    nc.compile()
    return nc


def _emit(nc, tc, xq, xk, xv, wq, wk, wv, wo, wf, kbias, qmask, out):
    from contextlib import ExitStack

    dt = mybir.dt
    Exp = mybir.ActivationFunctionType.Exp

    with ExitStack() as ctx:
        persist = ctx.enter_context(tc.tile_pool(name="persist", bufs=1))
        xin = ctx.enter_context(tc.tile_pool(name="xin", bufs=13))
        epool = ctx.enter_context(tc.tile_pool(name="e", bufs=10))
        osbpool = ctx.enter_context(tc.tile_pool(name="osb", bufs=4))
        onpool = ctx.enter_context(tc.tile_pool(name="on", bufs=4))
        denpool = ctx.enter_context(tc.tile_pool(name="den", bufs=4))
        rbcpool = ctx.enter_context(tc.tile_pool(name="rbc", bufs=4))
        outpool = ctx.enter_context(tc.tile_pool(name="outp", bufs=3))
        # PSUM budget (8 banks): sc 2x[128,1024]=4, o 2x[128,512]=2,
        # mm 2x[128,512]=2.
        ps_sc = ctx.enter_context(tc.tile_pool(name="ps_sc", bufs=2, space="PSUM"))
        ps_o = ctx.enter_context(tc.tile_pool(name="ps_o", bufs=2, space="PSUM"))
        ps_mm = ctx.enter_context(tc.tile_pool(name="ps_mm", bufs=2, space="PSUM"))

        # ---- persistent SBUF tensors ----
        qT = persist.tile([128, 2, S], dt.bfloat16, tag="qT")
        kT = persist.tile([128, 2, S], dt.bfloat16, tag="kT")
        vsb = persist.tile([128, NTT, HG, 65], dt.bfloat16, tag="v")
        wq_sb = persist.tile([128, NE + 1, HG * D], dt.bfloat16, tag="wq")
        wk_sb = persist.tile([128, NE + 1, HG * D], dt.bfloat16, tag="wk")
        wv_sb = persist.tile([128, NE + 1, HG * D], dt.bfloat16, tag="wv")
        wo_sb = persist.tile([D, HG * O], dt.bfloat16, tag="wo")
        wf_sb = persist.tile([128, 2, E], dt.bfloat16, tag="wf")
        kb_sb = persist.tile([TT, NTT], dt.float32, tag="kb")
        qm_sb = persist.tile([65, S], dt.float32, tag="qm")
        ones_row = persist.tile([1, S], dt.bfloat16, tag="ones_row")
        ones64 = persist.tile([65, D], dt.float32, tag="ones64")
        xT = persist.tile([128, 2, S], dt.bfloat16, tag="xT")

        # ---- weight/mask loads + constants ----
        for w_dram, w_sb in ((wq, wq_sb), (wk, wk_sb), (wv, wv_sb)):
            for c in range(NE):
                nc.sync.dma_start(out=w_sb[:, c, :], in_=w_dram[128 * c : 128 * (c + 1), :])
            nc.sync.dma_start(out=w_sb[0:1, NE, :], in_=w_dram[E : E + 1, :])
        nc.sync.dma_start(out=wo_sb[:, :], in_=wo[:, :])
        for cc in range(2):
            nc.sync.dma_start(out=wf_sb[:, cc, :], in_=wf[128 * cc : 128 * (cc + 1), :])
        nc.sync.dma_start(out=kb_sb[:, :], in_=kbias[:, :])
        nc.sync.dma_start(out=qm_sb[64:65, :], in_=qmask[0:1, :])
        nc.vector.memset(ones_row[:, :], 1.0)
        nc.vector.memset(ones64[64:65, :], 1.0)
        nc.vector.memset(vsb[:, :, :, 64:65], 1.0)

        copy_flip = [0]

        def psum_copy(dst_ap, src_ap):
            # Alternate PSUM->SBUF drains between DVE and ACT to balance load.
            if copy_flip[0] % 2 == 0:
                nc.vector.tensor_copy(dst_ap, src_ap)
            else:
                nc.scalar.copy(dst_ap, src_ap)
            copy_flip[0] += 1

        def proj_lhsT(xtiles, c, col_slice):
            # lhsT for projection chunk c: input chunk (K=128) or the
            # appended ones row (K=1) that adds the bias row of W_aug.
            if c < NE:
                return xtiles[c][:, col_slice]
            return ones_row[0:1, col_slice]

        def proj_rhs(w_sb, c, col_slice):
            if c < NE:
                return w_sb[:, c, col_slice]
            return w_sb[0:1, NE, col_slice]

        # ---- projection: v ([t, d], t on partitions) ----
        xtiles_v = []
        for c in range(NE):
            t = xin.tile([128, S], dt.bfloat16, tag="xin", name=f"xv{c}")
            nc.gpsimd.dma_start(out=t[:, :], in_=xv[128 * c : 128 * (c + 1), :])
            xtiles_v.append(t)
        for half in range(2):
            # 8 t-tiles per half; four [128,256] accumulation regions per
            # [128,1024] sc slot.
            vps = [
                ps_sc.tile([128, 1024], dt.float32, tag="sc", name=f"vps{half}_0"),
                ps_sc.tile([128, 1024], dt.float32, tag="sc", name=f"vps{half}_1"),
            ]
            for c in range(NE + 1):
                for j in range(8):
                    jt = 8 * half + j
                    tl = slice(TT * jt, TT * (jt + 1))
                    reg = vps[j // 4][:, 256 * (j % 4) : 256 * (j % 4 + 1)]
                    nc.tensor.matmul(
                        reg,
                        proj_lhsT(xtiles_v, c, tl),
                        proj_rhs(wv_sb, c, slice(0, HG * D)),
                        start=(c == 0),
                        stop=(c == NE),
                    )
            for j in range(8):
                jt = 8 * half + j
                reg = vps[j // 4][:, 256 * (j % 4) : 256 * (j % 4 + 1)]
                psum_copy(vsb[:, jt, :, 0:64], reg)

        # ---- projections: q^T and k^T ([d, s], d on partitions) ----
        # chunk-outer over 8 live accumulation groups (2 d-halves x 4
        # s-blocks) so the PE consumes streamed chunks at DMA pace.
        for x_dram, w_sb, dst, nm in ((xq, wq_sb, qT, "q"), (xk, wk_sb, kT, "k")):
            xtiles = []
            for c in range(NE):
                t = xin.tile([128, S], dt.bfloat16, tag="xin", name=f"x{nm}{c}")
                nc.gpsimd.dma_start(out=t[:, :], in_=x_dram[128 * c : 128 * (c + 1), :])
                xtiles.append(t)
            grp = {}
            sc0 = ps_sc.tile([128, 1024], dt.float32, tag="sc", name=f"{nm}ps0")
            sc1 = ps_sc.tile([128, 1024], dt.float32, tag="sc", name=f"{nm}ps1")
            o0 = ps_o.tile([128, SBLK], dt.float32, tag="o", name=f"{nm}ps2")
            o1 = ps_o.tile([128, SBLK], dt.float32, tag="o", name=f"{nm}ps3")
            m0 = ps_mm.tile([128, SBLK], dt.float32, tag="mm", name=f"{nm}ps4")
            m1 = ps_mm.tile([128, SBLK], dt.float32, tag="mm", name=f"{nm}ps5")
            grp[(0, 0)] = sc0[:, 0:512]
            grp[(0, 1)] = sc0[:, 512:1024]
            grp[(0, 2)] = sc1[:, 0:512]
            grp[(0, 3)] = sc1[:, 512:1024]
            grp[(1, 0)] = o0[:, :]
            grp[(1, 1)] = o1[:, :]
            grp[(1, 2)] = m0[:, :]
            grp[(1, 3)] = m1[:, :]
            for c in range(NE + 1):
                for p in range(2):
                    dsl = slice(128 * p, 128 * (p + 1))
                    for sb in range(NSB):
                        sl = slice(SBLK * sb, SBLK * (sb + 1))
                        nc.tensor.matmul(
                            grp[(p, sb)],
                            proj_rhs(w_sb, c, dsl),
                            proj_lhsT(xtiles, c, sl),
                            start=(c == 0),
                            stop=(c == NE),
                        )
            for p in range(2):
                for sb in range(NSB):
                    sl = slice(SBLK * sb, SBLK * (sb + 1))
                    psum_copy(dst[:, p, sl], grp[(p, sb)])

        # ---- attention ----
        # Per (pair p, s-block sb): scores^T for both heads into one
        # [128,1024] psum tile -> one exp -> per-head o_un^T accumulation.
        # The normalize/Wo tail of the previous block is spread across tt
        # milestones of the current block so the PE never stalls on DVE.
        pending = [None]
        tail_state = {}

        def tail_den(prev, hi):
            p, sb, osbs = prev
            sl = slice(SBLK * sb, SBLK * (sb + 1))
            den = denpool.tile([65, SBLK], dt.float32, tag="den", name=f"den{p}_{sb}_{hi}")
            nc.vector.tensor_copy(den[64:65, :], osbs[hi][64:65, :])
            nc.vector.reciprocal(den[64:65, :], den[64:65, :])
            nc.vector.tensor_mul(den[64:65, :], den[64:65, :], qm_sb[64:65, sl])
            return den

        def tail_rbc(den, tag):
            rb = ps_mm.tile([128, SBLK], dt.float32, tag="mm", name=f"rb{tag}")
            nc.tensor.matmul(
                rb[0:64, :], ones64[64:65, 0:64], den[64:65, :],
                start=True, stop=True,
            )
            return rb

        def tail_norm(prev, hi, rb, tag):
            osbs = prev[2]
            rbc = rbcpool.tile([64, SBLK], dt.float32, tag="rbc", name=f"rbc{tag}")
            nc.vector.tensor_copy(rbc[:, :], rb[0:64, :])
            onT = onpool.tile([64, SBLK], dt.bfloat16, tag="on", name=f"on{tag}")
            nc.vector.tensor_mul(onT[:, :], osbs[hi][0:64, :], rbc[:, :])
            return onT

        def tail_wo(prev, onTs):
            p, sb, _ = prev
            sl = slice(SBLK * sb, SBLK * (sb + 1))
            o2 = ps_mm.tile([128, SBLK], dt.float32, tag="mm", name=f"o2_{p}_{sb}")
            for hi in (0, 1):
                h = 2 * p + hi
                nc.tensor.matmul(
                    o2[64 * hi : 64 * (hi + 1), :],
                    wo_sb[:, O * h : O * (h + 1)],
                    onTs[hi][:, :],
                    start=True,
                    stop=True,
                    tile_position=(0, 64 * hi),
                )
            nc.vector.tensor_copy(xT[:, p, sl], o2[:, :])

        def run_tail_milestone(tt):
            prev = pending[0]
            if prev is None:
                return
            p, sb, _ = prev
            tg = f"{p}_{sb}"
            if tt == 2:
                tail_state["denA"] = tail_den(prev, 0)
            elif tt == 4:
                tail_state["rbA"] = tail_rbc(tail_state["denA"], "A" + tg)
            elif tt == 6:
                tail_state["onA"] = tail_norm(prev, 0, tail_state["rbA"], "A" + tg)
            elif tt == 8:
                tail_state["denB"] = tail_den(prev, 1)
            elif tt == 10:
                tail_state["rbB"] = tail_rbc(tail_state["denB"], "B" + tg)
            elif tt == 12:
                tail_state["onB"] = tail_norm(prev, 1, tail_state["rbB"], "B" + tg)
            elif tt == 14:
                tail_wo(prev, (tail_state["onA"], tail_state["onB"]))
                tail_state.clear()
                pending[0] = None

        def av(o_ps, p, tt, et, start, stop):
            for hi in (0, 1):
                h = 2 * p + hi
                nc.tensor.matmul(
                    o_ps[hi][0:65, :],
                    vsb[:, tt, h, 0:65],
                    et[:, 512 * hi : 512 * (hi + 1)],
                    start=start,
                    stop=stop,
                )

        for p in range(2):
            for sb in range(NSB):
                sl = slice(SBLK * sb, SBLK * (sb + 1))
                o_ps = [
                    ps_o.tile([128, SBLK], dt.float32, tag="o", name=f"ops{p}_{sb}_{hi}")
                    for hi in (0, 1)
                ]
                etiles = [None] * NTT
                for tt in range(NTT):
                    tl = slice(TT * tt, TT * (tt + 1))
                    sc = ps_sc.tile([128, 1024], dt.float32, tag="sc", name=f"sc{p}_{sb}_{tt}")
                    for hi in (0, 1):
                        base = 64 * hi
                        nc.tensor.matmul(
                            sc[:, 512 * hi : 512 * (hi + 1)],
                            kT[base : base + 64, p, tl],
                            qT[base : base + 64, p, sl],
                            start=True,
                            stop=True,
                            tile_position=(base, 0),
                        )
                    et = epool.tile([128, 1024], dt.bfloat16, tag="e", name=f"e{p}_{sb}_{tt}")
                    nc.scalar.activation(
                        et[:, :], sc[:, :], Exp,
                        bias=kb_sb[:, tt : tt + 1], scale=0.125,
                    )
                    etiles[tt] = et
                    if tt > 0:
                        av(o_ps, p, tt - 1, etiles[tt - 1], tt - 1 == 0, False)
                    run_tail_milestone(tt)
                av(o_ps, p, NTT - 1, etiles[NTT - 1], False, True)
                # stage o_un^T out of PSUM so the o slots free early and the
                # tail works from SBUF
                osbs = [
                    osbpool.tile([65, SBLK], dt.float32, tag="osb", name=f"osb{p}_{sb}_{hi}")
                    for hi in (0, 1)
                ]
                for hi in (0, 1):
                    nc.vector.tensor_copy(osbs[hi][:, :], o_ps[hi][0:65, :])
                pending[0] = (p, sb, osbs)

        # drain the last block's tail
        for tt in (2, 4, 6, 8, 10, 12, 14):
            run_tail_milestone(tt)

        # ---- final linear: partial = x^T.T @ Wf ----
        for st in range(NST):
            ssl = slice(128 * st, 128 * (st + 1))
            ps0 = ps_mm.tile([128, SBLK], dt.float32, tag="mm", name=f"wf{st}_0")
            ps1 = ps_mm.tile([128, SBLK], dt.float32, tag="mm", name=f"wf{st}_1")
            nc.tensor.matmul(ps0[:, :], xT[:, 0, ssl], wf_sb[:, 0, 0:512], start=True, stop=False)
            nc.tensor.matmul(ps1[:, :], xT[:, 0, ssl], wf_sb[:, 0, 512:1024], start=True, stop=False)
            nc.tensor.matmul(ps0[:, :], xT[:, 1, ssl], wf_sb[:, 1, 0:512], start=False, stop=True)
            nc.tensor.matmul(ps1[:, :], xT[:, 1, ssl], wf_sb[:, 1, 512:1024], start=False, stop=True)
            ot = outpool.tile([128, E], dt.float32, tag="outp", name=f"ot{st}")
            nc.vector.tensor_copy(ot[:, 0:512], ps0[:, :])
            nc.scalar.copy(ot[:, 512:1024], ps1[:, :])
            nc.sync.dma_start(out=out[ssl, :], in_=ot[:, :])


def _get_program():
    global _PROGRAM
    if _PROGRAM is None:
        _PROGRAM = _build_program()
    return _PROGRAM


def make_core_inputs(query, key, value, query_mask, key_mask, Wq, bq, Wk, bk, Wv, bv):
    """Per-core input maps (host-side sharding + layout + bf16 cast)."""
    in_maps = []
    for c in range(N_CORES):
        b, hg = c // 4, c % 4
        hs = slice(HG * hg, HG * (hg + 1))
        m = {
            "xq": np.ascontiguousarray(query[b].T).astype(BF16),
            "xk": np.ascontiguousarray(key[b].T).astype(BF16),
            "xv": np.ascontiguousarray(value[b].T).astype(BF16),
            # (kmask-1)*30000 gives 0 for kept keys, -30000 for masked ones;
            # laid out as [TT, NTT] columns so kbias[:, tt] is the
            # per-partition bias for t-tile tt.
            "kbias": np.ascontiguousarray(
                ((key_mask[b].astype(np.float32) - 1.0) * -NEG).reshape(NTT, TT).T
            ).astype(np.float32),
            "qmask": query_mask[b].astype(np.float32).reshape(1, S),
        }
        for name, W, bias in (("wq", Wq, bq), ("wk", Wk, bk), ("wv", Wv, bv)):
            w_aug = np.empty((E + 1, HG * D), np.float32)
            w_aug[:E] = W[hs].transpose(1, 0, 2).reshape(E, HG * D)
            w_aug[E] = bias[hs].reshape(HG * D)
            m[name] = w_aug.astype(BF16)
        in_maps.append(m)
    return in_maps


def kernel(**inputs):
    global LAST_RESULTS
    inp = {k: np.asarray(v) for k, v in inputs.items()}
    query, key, value = inp["query"], inp["key"], inp["value"]
    query_mask, key_mask = inp["query_mask"], inp["key_mask"]
    Wq, bq = inp["Wq"], inp["bq"]
    Wk, bk = inp["Wk"], inp["bk"]
    Wv, bv = inp["Wv"], inp["bv"]
    Wo, bo = inp["Wo"], inp["bo"]
    Wf, bf = inp["Wf"], inp["bf"]

    in_maps = make_core_inputs(
        query, key, value, query_mask, key_mask, Wq, bq, Wk, bk, Wv, bv
    )
    for c in range(N_CORES):
        hg = c % 4
        hs = slice(HG * hg, HG * (hg + 1))
        # wo: [D, HG*O], head h cols = Wo[head] (lhsT per head)
        in_maps[c]["wo"] = (
            Wo[hs].transpose(1, 0, 2).reshape(D, HG * O).astype(BF16)
        )
        in_maps[c]["wf"] = Wf[HG * O * hg : HG * O * (hg + 1), :].astype(BF16)

    nc = _get_program()
    res = run_bass_kernel_spmd(nc, in_maps, list(range(N_CORES)))
    LAST_RESULTS = res

    out = np.zeros((B, S, E), np.float32)
    for c in range(N_CORES):
        out[c // 4] += res.results[c]["out"]
    # fold per-head output bias and final bias: (bo_flat @ Wf + bf)
    bias = (
        bo.reshape(1, H * O).astype(np.float64) @ Wf.astype(np.float64)
    ).reshape(E) + bf.astype(np.float64)
    out += bias.astype(np.float32)
    return out
